# revision 7
# baseline (speedup 1.0000x reference)
"""Trainium2 Bass kernel for CustomAttention (qkv -> per-head LN on q,k -> SDPA -> proj).

Sharding: 8 cores = 2 batches x 4 head-groups (3 heads each).

v2 layout vs v1:
- Per-head phase B (qkv+LN), emitted B0,B1,C0,B2,C1,C2,proj so head h+1's
  projection/LN overlaps head h's attention on otherwise-idle engines.
- q/k transposes moved from PE to the DMA xbar (dma_start_transpose),
  batched per head; removes PE transpose rows and their PSUM->SBUF copies.
- ACT engine runs (almost) only the softmax exp, on [128,1536] PSUM tiles.
- LN stats/elementwise batched across all 32 n-blocks per head; mu/rstd
  subtraction+scale on GPSIMD(Pool), gamma/beta on DVE in bf16.
- Output projection in a final phase; PSUM budget: scores 2x3 banks +
  PV accumulators 2x1.
"""

import os
import sys
from functools import lru_cache

import numpy as np

for _p in ("/opt/trn_rl_repo", os.path.expanduser("~/.axon_site/_ro/trn_rl_repo")):
    if os.path.isdir(_p) and _p not in sys.path:
        sys.path.insert(0, _p)

import concourse.bass as bass
import concourse.mybir as mybir
from concourse import bacc
import concourse.tile as tile
from concourse.masks import make_identity

F32 = mybir.dt.float32
F32R = mybir.dt.float32r
BF16 = mybir.dt.bfloat16
ALU = mybir.AluOpType
ACTF = mybir.ActivationFunctionType

H = 3          # heads per core
D = 64         # head dim
C = 768        # model dim
J = 3 * H * D  # qkv rows per core = 576
EPS = 1e-5
SCALE = D ** -0.5

# PV matmuls run this many 512-wide probability chunks behind the score
# matmuls so PE has work while ACT produces exp tiles.
SKEW = 6
GRP = 3        # score j-chunks per PSUM/exp tile


def r32(ap):
    return ap.bitcast(F32R)


def build_nc(N=4096):
    """One-core program; all 8 cores run it SPMD with different input data."""
    NB = N // 128          # n-blocks / j-chunks = 32
    IB = N // 512          # i-blocks = 8
    NHALF = NB // 2

    nc = bacc.Bacc("TRN2", target_bir_lowering=False, debug=False)
    x_t = nc.declare_dram_parameter("x_t", [C, N], BF16, isOutput=False)
    # host layout: [C, (h, q|k|v, 64)] = per-head column groups
    wqkv_t = nc.declare_dram_parameter("wqkv_t", [C, J], BF16, isOutput=False)
    projw_t = nc.declare_dram_parameter("projw_t", [H * D, C], F32, isOutput=False)
    gb = nc.declare_dram_parameter("gb", [4, D], F32, isOutput=False)
    out_p = nc.declare_dram_parameter("out_p", [N, C], F32, isOutput=True)

    with tile.TileContext(nc) as tc:
        with (
            tc.tile_pool(name="persist", bufs=1) as persist,
            tc.tile_pool(name="weights", bufs=1) as weights,
            tc.tile_pool(name="raw", bufs=2) as rawp,
            tc.tile_pool(name="stats", bufs=2) as statsp,
        ):
            # ---- persistent SBUF tensors ----
            # qT duplicated across both partition halves: rows 0:64 == 64:128
            qT = [persist.tile([128, N], BF16, tag=f"qT{h}", name=f"qT{h}") for h in range(H)]
            # kT stacked: rows 0:64 = j in [0,N/2), rows 64:128 = j in [N/2,N)
            kT = [persist.tile([128, N // 2], BF16, tag=f"kT{h}", name=f"kT{h}") for h in range(H)]
            # V augmented with a ones column (index 64) per j-chunk
            vA = [persist.tile([128, NB, 65], BF16, tag=f"vA{h}", name=f"vA{h}") for h in range(H)]
            # attention output, channel-major: ao1 rows = h0,h1; ao2 rows = h2
            ao1 = persist.tile([128, N], F32R, tag="ao1")
            ao2 = persist.tile([64, N], F32R, tag="ao2")

            for h in range(H):
                nc.vector.memset(vA[h][:, :, 64:65], 1.0)

            wq = weights.tile([128, 6, J], BF16, tag="wqkv")
            nc.sync.dma_start(
                wq[:], wqkv_t.rearrange("(ck p) j -> p ck j", p=128)
            )
            pw128 = weights.tile([128, C], F32R, tag="pw128")
            nc.sync.dma_start(pw128[:], projw_t[0:128, :].bitcast(F32R))
            pw64 = weights.tile([64, C], F32R, tag="pw64")
            nc.sync.dma_start(pw64[:], projw_t[128:192, :].bitcast(F32R))
            # gamma/beta broadcast across partitions: rows [gq*s, bq*s, gk, bk]
            gbt = weights.tile([128, 4, D], F32, tag="gb")
            nc.sync.dma_start(gbt[:], gb[None, :, :].to_broadcast([128, 4, D]))
            # bf16 [2, D] gamma (q-scaled, k) and beta for the LN elementwise
            gam2 = weights.tile([128, 2, D], BF16, tag="gam2")
            nc.vector.tensor_copy(gam2[:, 0, :], gbt[:, 0, :])
            nc.vector.tensor_copy(gam2[:, 1, :], gbt[:, 2, :])
            bet2 = weights.tile([128, 2, D], BF16, tag="bet2")
            nc.vector.tensor_copy(bet2[:, 0, :], gbt[:, 1, :])
            nc.vector.tensor_copy(bet2[:, 1, :], gbt[:, 3, :])
            epst = weights.tile([128, 1], F32, tag="epst")
            nc.vector.memset(epst[:], EPS)
            ident = weights.tile([128, 128], F32, tag="ident")
            make_identity(nc, ident[:])
            identb = weights.tile([128, 128], BF16, tag="identb")
            nc.vector.tensor_copy(identb[:], ident[:])

            def phase_b(h):
                """qkv projection + LN for head h; ends with DMA transposes
                filling qT[h], kT[h] (and vA[h] v-copies along the way)."""
                # raw/LN slab: [128, (q|k), nb, d] bf16, written in-place
                raw = rawp.tile([128, 2, NB, D], BF16, tag="raw", name=f"raw{h}")
                s1 = statsp.tile([128, 2, NB], F32, tag="s1")
                s2 = statsp.tile([128, 2, NB], F32, tag="s2")
                with (
                    tc.tile_pool(name=f"pB{h}", bufs=4) as pB,
                    tc.tile_pool(name=f"psB{h}", bufs=4, space="PSUM") as psB,
                ):
                    for nb in range(NB):
                        xt = pB.tile([128, 6, 128], BF16, tag="xt")
                        nc.sync.dma_start(
                            xt[:],
                            x_t.rearrange("(ck p) n -> p ck n", p=128)[
                                :, :, nb * 128 : (nb + 1) * 128
                            ],
                        )
                        ps = psB.tile([128, 192], F32, tag="qkvps")
                        for ck in range(6):
                            nc.tensor.matmul(
                                ps[:],
                                xt[:, ck, :],
                                wq[:, ck, h * 192 : (h + 1) * 192],
                                start=(ck == 0),
                                stop=(ck == 5),
                            )
                        # q,k -> raw slab (bf16); v -> vA
                        nc.vector.tensor_copy(
                            raw[:, :, nb, :],
                            ps[:, 0:128].rearrange("p (t d) -> p t d", t=2),
                        )
                        nc.vector.tensor_copy(vA[h][:, nb, 0:64], ps[:, 128:192])

                # ---- batched LN stats over all nb ----
                sq = rawp.tile([128, 2, NB, D], BF16, tag="sq", name=f"sq{h}")
                nc.vector.tensor_reduce(s1[:], raw[:], mybir.AxisListType.X, ALU.add)
                nc.vector.tensor_mul(sq[:], raw[:], raw[:])
                nc.vector.tensor_reduce(s2[:], sq[:], mybir.AxisListType.X, ALU.add)
                mu = statsp.tile([128, 2, NB], F32, tag="mu")
                nc.vector.tensor_scalar_mul(mu[:], s1[:], 1.0 / D)
                var = statsp.tile([128, 2, NB], F32, tag="var")
                nc.vector.tensor_scalar_mul(var[:], s2[:], 1.0 / D)
                musq = statsp.tile([128, 2, NB], F32, tag="musq")
                nc.vector.tensor_mul(musq[:], mu[:], mu[:])
                nc.vector.tensor_sub(var[:], var[:], musq[:])
                std = statsp.tile([128, 2, NB], F32, tag="std")
                nc.scalar.activation(std[:], var[:], ACTF.Sqrt, bias=epst[:])
                rstd = statsp.tile([128, 2, NB], F32, tag="rstd")
                nc.vector.reciprocal(rstd[:], std[:])
                # one Newton step: r <- r*(1.5 - 0.5*(var+eps)*r^2)
                nr = statsp.tile([128, 2, NB], F32, tag="nr")
                nc.vector.tensor_mul(nr[:], rstd[:], rstd[:])
                ve = statsp.tile([128, 2, NB], F32, tag="ve")
                nc.vector.tensor_scalar_add(ve[:], var[:], EPS)
                nc.vector.tensor_mul(nr[:], nr[:], ve[:])
                nc.vector.tensor_scalar(nr[:], nr[:], -0.5, 1.5, ALU.mult, ALU.add)
                nc.vector.tensor_mul(rstd[:], rstd[:], nr[:])

                # ---- batched LN elementwise (in place on raw) ----
                mu_b = statsp.tile([128, 2, NB], BF16, tag="mu_b")
                nc.vector.tensor_copy(mu_b[:], mu[:])
                rstd_b = statsp.tile([128, 2, NB], BF16, tag="rstd_b")
                nc.vector.tensor_copy(rstd_b[:], rstd[:])
                mu4 = mu_b[:, :, :, None].broadcast_to([128, 2, NB, D])
                rs4 = rstd_b[:, :, :, None].broadcast_to([128, 2, NB, D])
                nc.gpsimd.tensor_sub(raw[:], raw[:], mu4)
                nc.gpsimd.tensor_mul(raw[:], raw[:], rs4)
                g4 = gam2[:, :, None, :].broadcast_to([128, 2, NB, D])
                b4 = bet2[:, :, None, :].broadcast_to([128, 2, NB, D])
                nc.vector.tensor_mul(raw[:], raw[:], g4)
                nc.vector.tensor_add(raw[:], raw[:], b4)

                # ---- transposes on PE (tile_position places both q dup
                # halves and the stacked k halves without duplicate writes) ----
                with tc.tile_pool(name=f"psT{h}", bufs=4, space="PSUM") as psT:
                    for nb in range(NB):
                        blk = slice(nb * 128, (nb + 1) * 128)
                        pq = psT.tile([128, 128], BF16, tag="pq")
                        nc.tensor.transpose(pq[0:64, :], raw[:, 0, nb, :], identb[:])
                        nc.tensor.transpose(
                            pq[64:128, :], raw[:, 0, nb, :], identb[:],
                            tile_position=(0, 64),
                        )
                        nc.vector.tensor_copy(qT[h][:, blk], pq[:])
                        jh, cb = nb // NHALF, nb % NHALF
                        psl = slice(64 * jh, 64 * jh + 64)
                        pk = psT.tile([128, 128], BF16, tag="pk")
                        nc.tensor.transpose(
                            pk[psl, :], raw[:, 1, nb, :], identb[:],
                            tile_position=(0, 64 * jh),
                        )
                        nc.vector.tensor_copy(
                            kT[h][psl, cb * 128 : (cb + 1) * 128], pk[psl, :]
                        )

            def phase_c(h):
                """Full attention for head h."""
                with (
                    tc.tile_pool(name=f"pt{h}", bufs=4) as ptp,
                    tc.tile_pool(name=f"pCs{h}", bufs=4) as pCs,
                    tc.tile_pool(name=f"psS{h}", bufs=2, space="PSUM") as psS,
                    tc.tile_pool(name=f"psO{h}", bufs=2, space="PSUM") as psO,
                ):
                    ngrp = (NB + GRP - 1) // GRP
                    for ib in range(IB):
                        isl = slice(ib * 512, (ib + 1) * 512)
                        pso = psO.tile([65, 512], F32, tag="pso")
                        queue = []
                        n_pv = [0]

                        def emit_pv(pso=pso, queue=queue, n_pv=n_pv, h=h):
                            pt_half, jc = queue.pop(0)
                            nc.tensor.matmul(
                                pso[:],
                                vA[h][:, jc, :],
                                pt_half,
                                start=(n_pv[0] == 0),
                                stop=(n_pv[0] == NB - 1),
                            )
                            n_pv[0] += 1

                        for g in range(ngrp):
                            lo = g * GRP
                            hi = min(lo + GRP, NB)
                            w = hi - lo
                            ps = psS.tile([128, GRP, 512], F32, tag="st")
                            for s in range(w):
                                jc = lo + s
                                jh, cb = jc // NHALF, jc % NHALF
                                psl = slice(64 * jh, 64 * jh + 64)
                                nc.tensor.matmul(
                                    ps[:, s, :],
                                    kT[h][psl, cb * 128 : (cb + 1) * 128],
                                    qT[h][psl, isl],
                                    start=True,
                                    stop=True,
                                    tile_position=(64 * jh, 0),
                                )
                            pt = ptp.tile([128, GRP, 512], BF16, tag="pt")
                            nc.scalar.activation(
                                pt[:, 0:w, :], ps[:, 0:w, :], ACTF.Exp
                            )
                            for s in range(w):
                                queue.append((pt[:, s, :], lo + s))
                            while len(queue) > SKEW:
                                emit_pv()
                        while queue:
                            emit_pv()

                        rden_f = pCs.tile([1, 512], F32, tag="rden_f")
                        nc.vector.tensor_copy(rden_f[:], pso[64:65, :])
                        rden = pCs.tile([1, 512], F32, tag="rden")
                        nc.vector.reciprocal_approx_fast(rden[:], rden_f[:])
                        rb = pCs.tile([64, 512], F32, tag="rb")
                        nc.gpsimd.partition_broadcast(rb[:], rden[:])
                        if h == 0:
                            nc.vector.tensor_mul(ao1[0:64, isl], pso[0:64, :], rb[:])
                        elif h == 2:
                            nc.vector.tensor_mul(ao2[0:64, isl], pso[0:64, :], rb[:])
                        else:
                            stg = pCs.tile([64, 512], F32R, tag="stg")
                            nc.vector.tensor_mul(stg[:], pso[0:64, :], rb[:])
                            nc.sync.dma_start(ao1[64:128, isl], stg[:])

            # ---- emission order pipelines head h+1's B under head h's C ----
            phase_b(0)
            phase_b(1)
            phase_c(0)
            phase_b(2)
            phase_c(1)
            phase_c(2)

            # ---- output projection ----
            with (
                tc.tile_pool(name="pD", bufs=3) as pD,
                tc.tile_pool(name="psD1", bufs=2, space="PSUM") as psD1,
                tc.tile_pool(name="psD2", bufs=2, space="PSUM") as psD2,
            ):
                for nb in range(NB):
                    blk = slice(nb * 128, (nb + 1) * 128)
                    stage = pD.tile([128, C], F32, tag="stage")
                    for oc, osz, psD in ((0, 512, psD1), (512, 256, psD2)):
                        ps = psD.tile([128, osz], F32, tag=f"pd{osz}")
                        nc.tensor.matmul(
                            ps[:],
                            r32(ao1[:, blk]),
                            r32(pw128[:, oc : oc + osz]),
                            start=True,
                            stop=False,
                        )
                        nc.tensor.matmul(
                            ps[:],
                            r32(ao2[0:64, blk]),
                            r32(pw64[0:64, oc : oc + osz]),
                            start=False,
                            stop=True,
                        )
                        nc.vector.tensor_copy(stage[:, oc : oc + osz], ps[:])
                    nc.sync.dma_start(out_p[blk, :], stage[:])

    nc.compile()
    return nc


@lru_cache(maxsize=2)
def _built(N):
    nc = build_nc(N)
    return nc


def _prep_inputs(x, qkv_w, q_gamma, q_beta, k_gamma, k_beta, proj_w):
    x = np.asarray(x, np.float32)
    qkv_w = np.asarray(qkv_w, np.float32)
    proj_w = np.asarray(proj_w, np.float32)
    B = x.shape[0]
    import ml_dtypes
    xts = [np.ascontiguousarray(x[b].T).astype(ml_dtypes.bfloat16) for b in range(B)]
    gbs = []
    wqs = []
    pws = []
    for g in range(4):
        r = slice(192 * g, 192 * (g + 1))
        qg = qkv_w[0:768][r]       # [192, 768] q rows of this group's 3 heads
        kg = qkv_w[768:1536][r]
        vg = qkv_w[1536:2304][r]
        # per-head interleave: [q_h(64) | k_h(64) | v_h(64)] x 3 heads
        blocks = []
        for h in range(3):
            hs = slice(64 * h, 64 * (h + 1))
            blocks += [qg[hs], kg[hs], vg[hs]]
        wq_rows = np.concatenate(blocks, axis=0)   # [576, 768]
        wqs.append(np.ascontiguousarray(wq_rows.T).astype(ml_dtypes.bfloat16))
        pws.append(np.ascontiguousarray(proj_w[:, r].T))
        gbs.append(
            np.stack(
                [
                    np.asarray(q_gamma, np.float32) * SCALE,
                    np.asarray(q_beta, np.float32) * SCALE,
                    np.asarray(k_gamma, np.float32),
                    np.asarray(k_beta, np.float32),
                ]
            )
        )
    in_maps = []
    for core in range(8):
        b, g = core // 4, core % 4
        in_maps.append(
            {"x_t": xts[b], "wqkv_t": wqs[g], "projw_t": pws[g], "gb": gbs[g]}
        )
    return in_maps


def run_cores(in_maps, N, trace=False):
    from concourse.bass_utils import run_bass_kernel_spmd

    nc = _built(N)
    res = run_bass_kernel_spmd(nc, in_maps, list(range(8)), trace=trace)
    return res


def kernel(x, qkv_w, q_gamma, q_beta, k_gamma, k_beta, proj_w, proj_b):
    x = np.asarray(x, np.float32)
    N = x.shape[1]
    in_maps = _prep_inputs(x, qkv_w, q_gamma, q_beta, k_gamma, k_beta, proj_w)
    res = run_cores(in_maps, N)
    parts = [np.asarray(r["out_p"], np.float32) for r in res.results]
    out0 = parts[0] + parts[1] + parts[2] + parts[3]
    out1 = parts[4] + parts[5] + parts[6] + parts[7]
    out = np.stack([out0, out1]) + np.asarray(proj_b, np.float32)
    return out.astype(np.float32)


# revision 11
# speedup vs baseline: 1.0848x; 1.0848x over previous
"""Trainium2 Bass kernel for CustomAttention (qkv -> per-head LN on q,k -> SDPA -> proj).

Sharding: 8 cores = 2 batches x 4 head-groups (3 heads each).

v3 structure:
- Phase B for heads 0+1 together (384-row qkv matmuls, LDWEIGHTS-balanced),
  batched LN stats/elementwise per head, q written to a duplicated slab so
  each n-block needs one [128,128] PE transpose; k transposed straight from
  the LN slab with tile_position placing the stacked half.
- Head 2's qkv matmuls are interleaved into head 0's attention i-block loop,
  filling PE while ACT (exp) is the binding engine; its LN runs on DVE/Pool
  during attention.
- ACT runs only exp (on [128,1536] PSUM tiles) + tiny sqrt; all PSUM->SBUF
  copies are on DVE; mu/rstd LN passes on GPSIMD.
- Output projection in a final phase.
"""

import os
import sys
from functools import lru_cache

import numpy as np

for _p in ("/opt/trn_rl_repo", os.path.expanduser("~/.axon_site/_ro/trn_rl_repo")):
    if os.path.isdir(_p) and _p not in sys.path:
        sys.path.insert(0, _p)

import concourse.bass as bass
import concourse.mybir as mybir
from concourse import bacc
import concourse.tile as tile
from concourse.masks import make_identity

F32 = mybir.dt.float32
F32R = mybir.dt.float32r
BF16 = mybir.dt.bfloat16
ALU = mybir.AluOpType
ACTF = mybir.ActivationFunctionType

H = 3          # heads per core
D = 64         # head dim
C = 768        # model dim
J = 3 * H * D  # qkv rows per core = 576
EPS = 1e-5
SCALE = D ** -0.5

SKEW = 6       # PV matmuls lag scores by this many 512-wide chunks
GRP = 3        # score j-chunks per PSUM/exp tile


def r32(ap):
    return ap.bitcast(F32R)


def build_nc(N=4096):
    """One-core program; all 8 cores run it SPMD with different input data."""
    NB = N // 128          # n-blocks / j-chunks = 32
    IB = N // 512          # i-blocks = 8
    NHALF = NB // 2

    nc = bacc.Bacc("TRN2", target_bir_lowering=False, debug=False)
    x_t = nc.declare_dram_parameter("x_t", [C, N], BF16, isOutput=False)
    # host layout: [C, (h, q|k|v, 64)] = per-head column groups
    wqkv_t = nc.declare_dram_parameter("wqkv_t", [C, J], BF16, isOutput=False)
    projw_t = nc.declare_dram_parameter("projw_t", [H * D, C], F32, isOutput=False)
    gb = nc.declare_dram_parameter("gb", [4, D], F32, isOutput=False)
    out_p = nc.declare_dram_parameter("out_p", [N, C], F32, isOutput=True)

    with tile.TileContext(nc) as tc:
        with (
            tc.tile_pool(name="persist", bufs=1) as persist,
            tc.tile_pool(name="weights", bufs=1) as weights,
            tc.tile_pool(name="raw", bufs=2) as rawp,
            tc.tile_pool(name="stats", bufs=2) as statsp,
        ):
            # ---- persistent SBUF tensors ----
            # qT duplicated across both partition halves: rows 0:64 == 64:128
            qT = [persist.tile([128, N], BF16, tag=f"qT{h}", name=f"qT{h}") for h in range(H)]
            # kT stacked: rows 0:64 = j in [0,N/2), rows 64:128 = j in [N/2,N)
            kT = [persist.tile([128, N // 2], BF16, tag=f"kT{h}", name=f"kT{h}") for h in range(H)]
            # V augmented with a ones column (index 64) per j-chunk
            vA = [persist.tile([128, NB, 65], BF16, tag=f"vA{h}", name=f"vA{h}") for h in range(H)]
            # attention output, channel-major: ao1 rows = h0,h1; ao2 rows = h2
            ao1 = persist.tile([128, N], F32R, tag="ao1")
            ao2 = persist.tile([64, N], F32R, tag="ao2")

            for h in range(H):
                nc.vector.memset(vA[h][:, :, 64:65], 1.0)

            wq = weights.tile([128, 6, J], BF16, tag="wqkv")
            nc.sync.dma_start(
                wq[:], wqkv_t.rearrange("(ck p) j -> p ck j", p=128)
            )
            pw128 = weights.tile([128, C], F32R, tag="pw128")
            nc.sync.dma_start(pw128[:], projw_t[0:128, :].bitcast(F32R))
            pw64 = weights.tile([64, C], F32R, tag="pw64")
            nc.sync.dma_start(pw64[:], projw_t[128:192, :].bitcast(F32R))
            # gamma/beta broadcast across partitions: rows [gq*s, bq*s, gk, bk]
            gbt = weights.tile([128, 4, D], F32, tag="gb")
            nc.sync.dma_start(gbt[:], gb[None, :, :].to_broadcast([128, 4, D]))
            # bf16 [2, D] gamma (q-scaled, k) and beta for the LN elementwise
            gam2 = weights.tile([128, 2, D], BF16, tag="gam2")
            nc.vector.tensor_copy(gam2[:, 0, :], gbt[:, 0, :])
            nc.vector.tensor_copy(gam2[:, 1, :], gbt[:, 2, :])
            bet2 = weights.tile([128, 2, D], BF16, tag="bet2")
            nc.vector.tensor_copy(bet2[:, 0, :], gbt[:, 1, :])
            nc.vector.tensor_copy(bet2[:, 1, :], gbt[:, 3, :])
            epst = weights.tile([128, 1], F32, tag="epst")
            nc.vector.memset(epst[:], EPS)
            ident = weights.tile([128, 128], F32, tag="ident")
            make_identity(nc, ident[:])
            identb = weights.tile([128, 128], BF16, tag="identb")
            nc.vector.tensor_copy(identb[:], ident[:])

            raws = {}
            qdups = {}

            def ln_head(h, raw):
                """Batched LN stats + elementwise for one head's raw slab
                [128, 2, NB, D]; q result lands duplicated in qdups[h],
                k result in place at raw[:, 1]."""
                s1 = statsp.tile([128, 2, NB], F32, tag="s1")
                s2 = statsp.tile([128, 2, NB], F32, tag="s2")
                sq = rawp.tile([128, 2, NB, D], BF16, tag="sq", name=f"sq{h}")
                nc.vector.tensor_reduce(s1[:], raw[:], mybir.AxisListType.X, ALU.add)
                nc.vector.tensor_mul(sq[:], raw[:], raw[:])
                nc.vector.tensor_reduce(s2[:], sq[:], mybir.AxisListType.X, ALU.add)
                mu = statsp.tile([128, 2, NB], F32, tag="mu")
                nc.vector.tensor_scalar_mul(mu[:], s1[:], 1.0 / D)
                var = statsp.tile([128, 2, NB], F32, tag="var")
                nc.vector.tensor_scalar_mul(var[:], s2[:], 1.0 / D)
                musq = statsp.tile([128, 2, NB], F32, tag="musq")
                nc.vector.tensor_mul(musq[:], mu[:], mu[:])
                nc.vector.tensor_sub(var[:], var[:], musq[:])
                std = statsp.tile([128, 2, NB], F32, tag="std")
                nc.scalar.activation(std[:], var[:], ACTF.Sqrt, bias=epst[:])
                rstd = statsp.tile([128, 2, NB], F32, tag="rstd")
                nc.vector.reciprocal(rstd[:], std[:])
                # one Newton step: r <- r*(1.5 - 0.5*(var+eps)*r^2)
                nr = statsp.tile([128, 2, NB], F32, tag="nr")
                nc.vector.tensor_mul(nr[:], rstd[:], rstd[:])
                ve = statsp.tile([128, 2, NB], F32, tag="ve")
                nc.vector.tensor_scalar_add(ve[:], var[:], EPS)
                nc.vector.tensor_mul(nr[:], nr[:], ve[:])
                nc.vector.tensor_scalar(nr[:], nr[:], -0.5, 1.5, ALU.mult, ALU.add)
                nc.vector.tensor_mul(rstd[:], rstd[:], nr[:])

                mu_b = statsp.tile([128, 2, NB], BF16, tag="mu_b")
                nc.vector.tensor_copy(mu_b[:], mu[:])
                rstd_b = statsp.tile([128, 2, NB], BF16, tag="rstd_b")
                nc.vector.tensor_copy(rstd_b[:], rstd[:])
                mu4 = mu_b[:, :, :, None].broadcast_to([128, 2, NB, D])
                rs4 = rstd_b[:, :, :, None].broadcast_to([128, 2, NB, D])
                nc.gpsimd.tensor_sub(raw[:], raw[:], mu4)
                nc.gpsimd.tensor_mul(raw[:], raw[:], rs4)

                qdup = rawp.tile([128, NB, 2, D], BF16, tag="qdup", name=f"qdup{h}")
                gq = gam2[:, 0, None, :].broadcast_to([128, NB, D])
                bq = bet2[:, 0, None, :].broadcast_to([128, NB, D])
                nc.vector.tensor_mul(qdup[:, :, 0, :], raw[:, 0, :, :], gq)
                nc.vector.tensor_add(qdup[:, :, 0, :], qdup[:, :, 0, :], bq)
                nc.vector.tensor_copy(qdup[:, :, 1, :], qdup[:, :, 0, :])
                gk = gam2[:, 1, None, :].broadcast_to([128, NB, D])
                bk = bet2[:, 1, None, :].broadcast_to([128, NB, D])
                nc.vector.tensor_mul(raw[:, 1, :, :], raw[:, 1, :, :], gk)
                nc.vector.tensor_add(raw[:, 1, :, :], raw[:, 1, :, :], bk)
                raws[h] = raw
                qdups[h] = qdup

            def phase_b01():
                """qkv + LN for heads 0 and 1 (one pass over x)."""
                raw0 = rawp.tile([128, 2, NB, D], BF16, tag="raw", name="raw0")
                raw1 = rawp.tile([128, 2, NB, D], BF16, tag="raw", name="raw1")
                with (
                    tc.tile_pool(name="pB01", bufs=4) as pB,
                    tc.tile_pool(name="psB01", bufs=4, space="PSUM") as psB,
                ):
                    for nb in range(NB):
                        xt = pB.tile([128, 6, 128], BF16, tag="xt")
                        nc.sync.dma_start(
                            xt[:],
                            x_t.rearrange("(ck p) n -> p ck n", p=128)[
                                :, :, nb * 128 : (nb + 1) * 128
                            ],
                        )
                        ps = psB.tile([128, 384], F32, tag="qkvps")
                        for ck in range(6):
                            nc.tensor.matmul(
                                ps[:],
                                xt[:, ck, :],
                                wq[:, ck, 0:384],
                                start=(ck == 0),
                                stop=(ck == 5),
                            )
                        nc.vector.tensor_copy(
                            raw0[:, :, nb, :],
                            ps[:, 0:128].rearrange("p (t d) -> p t d", t=2),
                        )
                        nc.vector.tensor_copy(
                            raw1[:, :, nb, :],
                            ps[:, 192:320].rearrange("p (t d) -> p t d", t=2),
                        )
                        # v0 at cols 128:192, v1 at cols 320:384
                        vsrc = ps.rearrange("p (h x) -> p h x", h=2)[:, :, 128:192]
                        nc.vector.tensor_copy(vA[0][:, nb, 0:64], vsrc[:, 0, :])
                        nc.vector.tensor_copy(vA[1][:, nb, 0:64], vsrc[:, 1, :])
                ln_head(0, raw0)
                ln_head(1, raw1)

            def phase_b2_mm(pB, psB, nb_lo, nb_hi):
                """Head 2 qkv matmuls + psum->sbuf copies for a range of nb."""
                raw2 = raws[2]
                for nb in range(nb_lo, nb_hi):
                    xt = pB.tile([128, 6, 128], BF16, tag="xt2")
                    nc.sync.dma_start(
                        xt[:],
                        x_t.rearrange("(ck p) n -> p ck n", p=128)[
                            :, :, nb * 128 : (nb + 1) * 128
                        ],
                    )
                    ps = psB.tile([128, 192], F32, tag="qkvps2")
                    for ck in range(6):
                        nc.tensor.matmul(
                            ps[:],
                            xt[:, ck, :],
                            wq[:, ck, 384:576],
                            start=(ck == 0),
                            stop=(ck == 5),
                        )
                    nc.vector.tensor_copy(
                        raw2[:, :, nb, :],
                        ps[:, 0:128].rearrange("p (t d) -> p t d", t=2),
                    )
                    nc.vector.tensor_copy(vA[2][:, nb, 0:64], ps[:, 128:192])

            def phase_t(h):
                """PE transposes filling qT[h] and kT[h]."""
                raw, qdup = raws[h], qdups[h]
                with tc.tile_pool(name=f"psT{h}", bufs=4, space="PSUM") as psT:
                    for nb in range(NB):
                        blk = slice(nb * 128, (nb + 1) * 128)
                        pq = psT.tile([128, 128], BF16, tag="pq")
                        nc.tensor.transpose(
                            pq[:], qdup[:, nb, :, :].rearrange("p t d -> p (t d)"),
                            identb[:],
                        )
                        nc.vector.tensor_copy(qT[h][:, blk], pq[:])
                        jh, cb = nb // NHALF, nb % NHALF
                        psl = slice(64 * jh, 64 * jh + 64)
                        pk = psT.tile([128, 128], BF16, tag="pk")
                        nc.tensor.transpose(
                            pk[psl, :], raw[:, 1, nb, :], identb[:],
                            tile_position=(0, 64 * jh),
                        )
                        nc.vector.tensor_copy(
                            kT[h][psl, cb * 128 : (cb + 1) * 128], pk[psl, :]
                        )

            def phase_c(h, ib_hook=None, grp=GRP):
                """Full attention for head h; ib_hook(ib) emits filler PE work."""
                with (
                    tc.tile_pool(name=f"pt{h}", bufs=4) as ptp,
                    tc.tile_pool(name=f"pCs{h}", bufs=4) as pCs,
                    tc.tile_pool(name=f"psS{h}", bufs=2, space="PSUM") as psS,
                    tc.tile_pool(name=f"psO{h}", bufs=2, space="PSUM") as psO,
                ):
                    ngrp = (NB + grp - 1) // grp
                    for ib in range(IB):
                        isl = slice(ib * 512, (ib + 1) * 512)
                        pso = psO.tile([65, 512], F32, tag="pso")
                        queue = []
                        n_pv = [0]

                        def emit_pv(pso=pso, queue=queue, n_pv=n_pv, h=h):
                            pt_half, jc = queue.pop(0)
                            nc.tensor.matmul(
                                pso[:],
                                vA[h][:, jc, :],
                                pt_half,
                                start=(n_pv[0] == 0),
                                stop=(n_pv[0] == NB - 1),
                            )
                            n_pv[0] += 1

                        for g in range(ngrp):
                            lo = g * grp
                            hi = min(lo + grp, NB)
                            w = hi - lo
                            ps = psS.tile([128, grp, 512], F32, tag="st")
                            for s in range(w):
                                jc = lo + s
                                jh, cb = jc // NHALF, jc % NHALF
                                psl = slice(64 * jh, 64 * jh + 64)
                                nc.tensor.matmul(
                                    ps[:, s, :],
                                    kT[h][psl, cb * 128 : (cb + 1) * 128],
                                    qT[h][psl, isl],
                                    start=True,
                                    stop=True,
                                    tile_position=(64 * jh, 0),
                                )
                            pt = ptp.tile([128, GRP, 512], BF16, tag="pt")
                            nc.scalar.activation(
                                pt[:, 0:w, :], ps[:, 0:w, :], ACTF.Exp
                            )
                            for s in range(w):
                                queue.append((pt[:, s, :], lo + s))
                            while len(queue) > SKEW:
                                emit_pv()
                        while queue:
                            emit_pv()

                        rden_f = pCs.tile([1, 512], F32, tag="rden_f")
                        nc.vector.tensor_copy(rden_f[:], pso[64:65, :])
                        rden = pCs.tile([1, 512], F32, tag="rden")
                        nc.vector.reciprocal_approx_fast(rden[:], rden_f[:])
                        rb = pCs.tile([64, 512], F32, tag="rb")
                        nc.gpsimd.partition_broadcast(rb[:], rden[:])
                        if h == 0:
                            nc.vector.tensor_mul(ao1[0:64, isl], pso[0:64, :], rb[:])
                        elif h == 2:
                            nc.vector.tensor_mul(ao2[0:64, isl], pso[0:64, :], rb[:])
                        else:
                            stg = pCs.tile([64, 512], F32R, tag="stg")
                            nc.vector.tensor_mul(stg[:], pso[0:64, :], rb[:])
                            nc.sync.dma_start(ao1[64:128, isl], stg[:])
                        if ib_hook is not None:
                            ib_hook(ib)

            # ---- emission ----
            phase_b01()
            phase_t(0)
            # head 2 raw slab must exist before its interleaved matmuls
            raws[2] = rawp.tile([128, 2, NB, D], BF16, tag="raw", name="raw2")
            with (
                tc.tile_pool(name="pB2", bufs=4) as pB2,
                tc.tile_pool(name="psB2", bufs=2, space="PSUM") as psB2,
            ):
                phase_c(
                    0,
                    ib_hook=lambda ib: phase_b2_mm(pB2, psB2, ib * 4, ib * 4 + 4),
                    grp=2,
                )
            ln_head(2, raws[2])
            phase_t(1)
            phase_c(1)
            phase_t(2)
            phase_c(2)

            # ---- output projection ----
            with (
                tc.tile_pool(name="pD", bufs=3) as pD,
                tc.tile_pool(name="psD1", bufs=2, space="PSUM") as psD1,
                tc.tile_pool(name="psD2", bufs=2, space="PSUM") as psD2,
            ):
                for nb in range(NB):
                    blk = slice(nb * 128, (nb + 1) * 128)
                    stage = pD.tile([128, C], F32, tag="stage")
                    for oc, osz, psD in ((0, 512, psD1), (512, 256, psD2)):
                        ps = psD.tile([128, osz], F32, tag=f"pd{osz}")
                        nc.tensor.matmul(
                            ps[:],
                            r32(ao1[:, blk]),
                            r32(pw128[:, oc : oc + osz]),
                            start=True,
                            stop=False,
                        )
                        nc.tensor.matmul(
                            ps[:],
                            r32(ao2[0:64, blk]),
                            r32(pw64[0:64, oc : oc + osz]),
                            start=False,
                            stop=True,
                        )
                        nc.vector.tensor_copy(stage[:, oc : oc + osz], ps[:])
                    nc.sync.dma_start(out_p[blk, :], stage[:])

    nc.compile()
    return nc


@lru_cache(maxsize=2)
def _built(N):
    nc = build_nc(N)
    return nc


def _prep_inputs(x, qkv_w, q_gamma, q_beta, k_gamma, k_beta, proj_w):
    x = np.asarray(x, np.float32)
    qkv_w = np.asarray(qkv_w, np.float32)
    proj_w = np.asarray(proj_w, np.float32)
    B = x.shape[0]
    import ml_dtypes
    xts = [np.ascontiguousarray(x[b].T).astype(ml_dtypes.bfloat16) for b in range(B)]
    gbs = []
    wqs = []
    pws = []
    for g in range(4):
        r = slice(192 * g, 192 * (g + 1))
        qg = qkv_w[0:768][r]       # [192, 768] q rows of this group's 3 heads
        kg = qkv_w[768:1536][r]
        vg = qkv_w[1536:2304][r]
        # per-head interleave: [q_h(64) | k_h(64) | v_h(64)] x 3 heads
        blocks = []
        for h in range(3):
            hs = slice(64 * h, 64 * (h + 1))
            blocks += [qg[hs], kg[hs], vg[hs]]
        wq_rows = np.concatenate(blocks, axis=0)   # [576, 768]
        wqs.append(np.ascontiguousarray(wq_rows.T).astype(ml_dtypes.bfloat16))
        pws.append(np.ascontiguousarray(proj_w[:, r].T))
        gbs.append(
            np.stack(
                [
                    np.asarray(q_gamma, np.float32) * SCALE,
                    np.asarray(q_beta, np.float32) * SCALE,
                    np.asarray(k_gamma, np.float32),
                    np.asarray(k_beta, np.float32),
                ]
            )
        )
    in_maps = []
    for core in range(8):
        b, g = core // 4, core % 4
        in_maps.append(
            {"x_t": xts[b], "wqkv_t": wqs[g], "projw_t": pws[g], "gb": gbs[g]}
        )
    return in_maps


def run_cores(in_maps, N, trace=False):
    from concourse.bass_utils import run_bass_kernel_spmd

    nc = _built(N)
    res = run_bass_kernel_spmd(nc, in_maps, list(range(8)), trace=trace)
    return res


def kernel(x, qkv_w, q_gamma, q_beta, k_gamma, k_beta, proj_w, proj_b):
    x = np.asarray(x, np.float32)
    N = x.shape[1]
    in_maps = _prep_inputs(x, qkv_w, q_gamma, q_beta, k_gamma, k_beta, proj_w)
    res = run_cores(in_maps, N)
    parts = [np.asarray(r["out_p"], np.float32) for r in res.results]
    out0 = parts[0] + parts[1] + parts[2] + parts[3]
    out1 = parts[4] + parts[5] + parts[6] + parts[7]
    out = np.stack([out0, out1]) + np.asarray(proj_b, np.float32)
    return out.astype(np.float32)
